# revision 7
# baseline (speedup 1.0000x reference)
"""CAM (channel attention) module kernel for Trainium2, 8-core data-parallel.

Computes, per batch b (one batch per NeuronCore):
    q = x[b].reshape(C, N)                  # C=512, N=4096
    E = q @ q.T                             # [C, C], symmetric
    att = softmax(rowmax(E) - E, axis=-1)   # == exp(rowmin(E)-E)/rowsum
    out = gamma * (att @ q) + x[b]

v2 design (fp32 matmul on trn2 is 2-pass LOW_HIGH emulation, ~5.5x slower
than bf16 -> do all matmuls in bf16, keep the +x and final scaling in fp32):
  - q loaded fp32 [128, 4, 4096]; cast to bf16 on DVE/ACT in 16 chunks.
  - qT built with DMA xbar transposes (bf16) into [128, 4(c), 32(k), 128],
    no PE or DVE time spent on transposition.
  - per channel-tile i (fused pipeline, overlaps across i):
      energy: 32 matmuls (lhsT=qT[:,i,k,:], rhs=qT[:,:,k,:]) accum in PSUM
      softmax: rowmin (DVE) -> exp(mn-E) on ACT writing bf16 att +
               fused row-sum (accum_out); rg = gamma/s kept per-partition
      attT: one DMA xbar transpose of att[:,i,:] -> [128, 4(j), 128]
      out: per 512-col chunk: 4 bf16 matmuls + one DVE
           scalar_tensor_tensor: out = (psum * rg) + x  (exact fp32 x-add)
      out DMA per chunk pair.
  - att is left unnormalized; gamma/s scaling rides the final DVE op, so
    gamma=0 gives out == x exactly.
"""

import sys

import numpy as np

for _p in ("/opt/trn_rl_repo",):
    if _p not in sys.path:
        sys.path.insert(0, _p)

B, C, H, W = 8, 512, 64, 64
N = H * W  # 4096
P = 128
CT = C // P  # 4 channel tiles
KT = N // P  # 32 spatial tiles
FD = 512  # matmul free-dim / PSUM bank width (fp32)
NCH = N // FD  # 8 output column chunks
LCH = 4  # input load chunks per c-tile
LW = N // LCH  # 1024

_CACHE = {}


def _build_bass():
    import concourse.mybir as mybir
    import concourse.tile as tile
    from concourse import bacc

    fp32 = mybir.dt.float32
    bf16 = mybir.dt.bfloat16
    AX = mybir.AxisListType.X
    ALU = mybir.AluOpType
    ACT_EXP = mybir.ActivationFunctionType.Exp

    nc = bacc.Bacc(None, target_bir_lowering=False, debug=False)
    x_d = nc.dram_tensor("x", [C, N], fp32, kind="ExternalInput")
    g_d = nc.dram_tensor("gamma", [1], fp32, kind="ExternalInput")
    o_d = nc.dram_tensor("out", [C, N], fp32, kind="ExternalOutput")

    with tile.TileContext(nc) as tc:
        with (
            tc.tile_pool(name="persist", bufs=1) as persist,
            tc.tile_pool(name="stats", bufs=4) as stats,
            tc.tile_pool(name="outp", bufs=4) as outp,
            tc.tile_pool(name="epsum", bufs=2, space="PSUM") as epsum,
            tc.tile_pool(name="opsum", bufs=4, space="PSUM") as opsum,
        ):
            gam = persist.tile([P, 1], fp32)
            nc.gpsimd.dma_start(out=gam, in_=g_d[:].to_broadcast((P, 1)))

            q = persist.tile([P, CT, N], fp32)
            q_bf = persist.tile([P, CT, N], bf16)
            qT = persist.tile([P, CT, KT, P], bf16)
            att = persist.tile([P, CT, C], bf16)
            attT = persist.tile([P, CT, CT, P], bf16)

            # load fp32, cast to bf16, xbar-transpose each chunk
            for h in range(LCH):
                for c in range(CT):
                    sl = slice(h * LW, (h + 1) * LW)
                    nc.sync.dma_start(out=q[:, c, sl], in_=x_d[c * P : (c + 1) * P, sl])
                    if (h + c) % 2 == 0:
                        nc.vector.tensor_copy(out=q_bf[:, c, sl], in_=q[:, c, sl])
                    else:
                        nc.scalar.copy(out=q_bf[:, c, sl], in_=q[:, c, sl])
                    ksl = slice(h * (LW // P), (h + 1) * (LW // P))
                    nc.sync.dma_start_transpose(
                        out=qT[:, c, ksl, :], in_=q_bf[:, c, sl]
                    )

            for i in range(CT):
                # ---- energy row-block i: E = q[i-block] @ q.T ----
                E = epsum.tile([P, C], fp32, name=f"E{i}", tag="E")
                for k in range(KT):
                    nc.tensor.matmul(
                        E,
                        lhsT=qT[:, i, k, :],
                        rhs=qT[:, :, k, :],
                        start=(k == 0),
                        stop=(k == KT - 1),
                    )

                # ---- softmax (unnormalized): att = exp(mn - E), s = rowsum ----
                mn = stats.tile([P, 1], fp32)
                nc.vector.tensor_reduce(out=mn, in_=E, axis=AX, op=ALU.min)
                s = stats.tile([P, 1], fp32)
                nc.scalar.activation(
                    out=att[:, i, :],
                    in_=E,
                    func=ACT_EXP,
                    bias=mn,
                    scale=-1.0,
                    accum_out=s,
                )
                rg = stats.tile([P, 1], fp32)
                nc.vector.reciprocal(out=rg, in_=s)
                nc.vector.tensor_mul(rg, rg, gam)

                # ---- attT slab i via xbar transpose ----
                nc.sync.dma_start_transpose(out=attT[:, i, :, :], in_=att[:, i, :])

                # ---- out row-block i ----
                ot = outp.tile([P, N], fp32, name="ot", tag="ot", bufs=2)
                for nh in range(NCH):
                    sl = slice(nh * FD, (nh + 1) * FD)
                    ops = opsum.tile([P, FD], fp32, name="ops", tag="ops")
                    for j in range(CT):
                        nc.tensor.matmul(
                            ops,
                            lhsT=attT[:, i, j, :],
                            rhs=q_bf[:, j, sl],
                            start=(j == 0),
                            stop=(j == CT - 1),
                        )
                    # out = (psum * gamma/s) + x, exact fp32 add of x
                    nc.vector.scalar_tensor_tensor(
                        out=ot[:, sl],
                        in0=ops,
                        scalar=rg,
                        in1=q[:, i, sl],
                        op0=ALU.mult,
                        op1=ALU.add,
                    )
                    if nh % 2 == 1:
                        osl = slice((nh - 1) * FD, (nh + 1) * FD)
                        nc.sync.dma_start(
                            out=o_d[i * P : (i + 1) * P, osl], in_=ot[:, osl]
                        )

    nc.compile()
    return nc


def _get_nc():
    if "nc" not in _CACHE:
        _CACHE["nc"] = _build_bass()
    return _CACHE["nc"]


def run(x, gamma, **run_kwargs):
    """Run on 8 cores; returns (results_list, BassKernelResults)."""
    from concourse.bass_utils import run_bass_kernel_spmd

    nc = _get_nc()
    x = np.ascontiguousarray(x, dtype=np.float32)
    gamma = np.ascontiguousarray(gamma, dtype=np.float32)
    in_maps = [
        {"x": np.ascontiguousarray(x[b].reshape(C, N)), "gamma": gamma}
        for b in range(B)
    ]
    res = run_bass_kernel_spmd(nc, in_maps, core_ids=list(range(B)), **run_kwargs)
    out = np.stack([r["out"] for r in res.results]).reshape(B, C, H, W)
    return out, res


def kernel(x, gamma):
    out, _ = run(x, gamma)
    return out.astype(np.float32)


# revision 9
# speedup vs baseline: 1.1711x; 1.1711x over previous
"""CAM (channel attention) module kernel for Trainium2, 8-core data-parallel.

Computes, per batch b (one batch per NeuronCore):
    q = x[b].reshape(C, N)                  # C=512, N=4096
    E = q @ q.T                             # [C, C], symmetric
    att = softmax(rowmax(E) - E, axis=-1)   # == exp(rowmin(E)-E)/rowsum
    out = gamma * (att @ q) + x[b]

v2 design (fp32 matmul on trn2 is 2-pass LOW_HIGH emulation, ~5.5x slower
than bf16 -> do all matmuls in bf16, keep the +x and final scaling in fp32):
  - q loaded fp32 [128, 4, 4096]; cast to bf16 on DVE/ACT in 16 chunks.
  - qT built with DMA xbar transposes (bf16) into [128, 4(c), 32(k), 128],
    no PE or DVE time spent on transposition.
  - per channel-tile i (fused pipeline, overlaps across i):
      energy: 32 matmuls (lhsT=qT[:,i,k,:], rhs=qT[:,:,k,:]) accum in PSUM
      softmax: rowmin (DVE) -> exp(mn-E) on ACT writing bf16 att +
               fused row-sum (accum_out); rg = gamma/s kept per-partition
      attT: one DMA xbar transpose of att[:,i,:] -> [128, 4(j), 128]
      out: per 512-col chunk: 4 bf16 matmuls + one DVE
           scalar_tensor_tensor: out = (psum * rg) + x  (exact fp32 x-add)
      out DMA per chunk pair.
  - att is left unnormalized; gamma/s scaling rides the final DVE op, so
    gamma=0 gives out == x exactly.
"""

import sys

import numpy as np

for _p in ("/opt/trn_rl_repo",):
    if _p not in sys.path:
        sys.path.insert(0, _p)

B, C, H, W = 8, 512, 64, 64
N = H * W  # 4096
P = 128
CT = C // P  # 4 channel tiles
KT = N // P  # 32 spatial tiles
FD = 512  # matmul free-dim / PSUM bank width (fp32)
NCH = N // FD  # 8 output column chunks
LCH = 4  # input load chunks per c-tile
LW = N // LCH  # 1024

_CACHE = {}


def _build_bass():
    import concourse.mybir as mybir
    import concourse.tile as tile
    from concourse import bacc

    fp32 = mybir.dt.float32
    bf16 = mybir.dt.bfloat16
    AX = mybir.AxisListType.X
    ALU = mybir.AluOpType
    ACT_EXP = mybir.ActivationFunctionType.Exp

    nc = bacc.Bacc(None, target_bir_lowering=False, debug=False)
    x_d = nc.dram_tensor("x", [C, N], fp32, kind="ExternalInput")
    g_d = nc.dram_tensor("gamma", [1], fp32, kind="ExternalInput")
    o_d = nc.dram_tensor("out", [C, N], fp32, kind="ExternalOutput")

    with tile.TileContext(nc) as tc:
        with (
            tc.tile_pool(name="persist", bufs=1) as persist,
            tc.tile_pool(name="stats", bufs=4) as stats,
            tc.tile_pool(name="outp", bufs=4) as outp,
            tc.tile_pool(name="epsum", bufs=2, space="PSUM") as epsum,
            tc.tile_pool(name="opsum", bufs=4, space="PSUM") as opsum,
        ):
            gam = persist.tile([P, 1], fp32)
            nc.gpsimd.dma_start(out=gam, in_=g_d[:].to_broadcast((P, 1)))

            q = persist.tile([P, CT, N], fp32)
            q_bf = persist.tile([P, CT, N], bf16)
            qT = persist.tile([P, CT, KT, P], bf16)
            att = persist.tile([P, CT, C], bf16)
            attT = persist.tile([P, CT, CT, P], bf16)

            # load fp32, cast to bf16, xbar-transpose each chunk.
            # loads ride the gpsimd(SWDGE)/scalar(ACT-HWDGE) rings so the
            # sync(SP-HWDGE) ring carries only the xbar transposes — a
            # transpose waiting on its cast must not block later loads
            # (HWDGE is FIFO per issuing engine).
            for h in range(LCH):
                for c in range(CT):
                    sl = slice(h * LW, (h + 1) * LW)
                    ld = nc.gpsimd if (h + c) % 2 == 0 else nc.scalar
                    ld.dma_start(out=q[:, c, sl], in_=x_d[c * P : (c + 1) * P, sl])
                    nc.vector.tensor_copy(out=q_bf[:, c, sl], in_=q[:, c, sl])
                    ksl = slice(h * (LW // P), (h + 1) * (LW // P))
                    nc.sync.dma_start_transpose(
                        out=qT[:, c, ksl, :], in_=q_bf[:, c, sl]
                    )

            for i in range(CT):
                # ---- energy row-block i: E = q[i-block] @ q.T ----
                E = epsum.tile([P, C], fp32, name=f"E{i}", tag="E")
                for k in range(KT):
                    nc.tensor.matmul(
                        E,
                        lhsT=qT[:, i, k, :],
                        rhs=qT[:, :, k, :],
                        start=(k == 0),
                        stop=(k == KT - 1),
                    )

                # ---- softmax (unnormalized): att = exp(mn - E), s = rowsum ----
                mn = stats.tile([P, 1], fp32)
                nc.vector.tensor_reduce(out=mn, in_=E, axis=AX, op=ALU.min)
                s = stats.tile([P, 1], fp32)
                nc.scalar.activation(
                    out=att[:, i, :],
                    in_=E,
                    func=ACT_EXP,
                    bias=mn,
                    scale=-1.0,
                    accum_out=s,
                )
                rg = stats.tile([P, 1], fp32)
                nc.vector.reciprocal(out=rg, in_=s)
                nc.vector.tensor_mul(rg, rg, gam)

                # ---- attT slab i via xbar transpose ----
                nc.sync.dma_start_transpose(out=attT[:, i, :, :], in_=att[:, i, :])

                # ---- out row-block i ----
                ot = outp.tile([P, N], fp32, name="ot", tag="ot", bufs=2)
                for nh in range(NCH):
                    sl = slice(nh * FD, (nh + 1) * FD)
                    ops = opsum.tile([P, FD], fp32, name="ops", tag="ops")
                    for j in range(CT):
                        nc.tensor.matmul(
                            ops,
                            lhsT=attT[:, i, j, :],
                            rhs=q_bf[:, j, sl],
                            start=(j == 0),
                            stop=(j == CT - 1),
                        )
                    # out = (psum * gamma/s) + x, exact fp32 add of x
                    nc.vector.scalar_tensor_tensor(
                        out=ot[:, sl],
                        in0=ops,
                        scalar=rg,
                        in1=q[:, i, sl],
                        op0=ALU.mult,
                        op1=ALU.add,
                    )
                    if nh % 4 == 3:
                        osl = slice((nh - 3) * FD, (nh + 1) * FD)
                        nc.gpsimd.dma_start(
                            out=o_d[i * P : (i + 1) * P, osl], in_=ot[:, osl]
                        )

    nc.compile()
    return nc


def _get_nc():
    if "nc" not in _CACHE:
        _CACHE["nc"] = _build_bass()
    return _CACHE["nc"]


def run(x, gamma, **run_kwargs):
    """Run on 8 cores; returns (results_list, BassKernelResults)."""
    from concourse.bass_utils import run_bass_kernel_spmd

    nc = _get_nc()
    x = np.ascontiguousarray(x, dtype=np.float32)
    gamma = np.ascontiguousarray(gamma, dtype=np.float32)
    in_maps = [
        {"x": np.ascontiguousarray(x[b].reshape(C, N)), "gamma": gamma}
        for b in range(B)
    ]
    res = run_bass_kernel_spmd(nc, in_maps, core_ids=list(range(B)), **run_kwargs)
    out = np.stack([r["out"] for r in res.results]).reshape(B, C, H, W)
    return out, res


def kernel(x, gamma):
    out, _ = run(x, gamma)
    return out.astype(np.float32)


# revision 11
# speedup vs baseline: 1.2727x; 1.0867x over previous
"""CAM (channel attention) module kernel for Trainium2, 8-core data-parallel.

Computes, per batch b (one batch per NeuronCore):
    q = x[b].reshape(C, N)                  # C=512, N=4096
    E = q @ q.T                             # [C, C], symmetric
    att = softmax(rowmax(E) - E, axis=-1)   # == exp(rowmin(E)-E)/rowsum
    out = gamma * (att @ q) + x[b]

v2 design (fp32 matmul on trn2 is 2-pass LOW_HIGH emulation, ~5.5x slower
than bf16 -> do all matmuls in bf16, keep the +x and final scaling in fp32):
  - q loaded fp32 [128, 4, 4096]; cast to bf16 on DVE/ACT in 16 chunks.
  - qT built with DMA xbar transposes (bf16) into [128, 4(c), 32(k), 128],
    no PE or DVE time spent on transposition.
  - per channel-tile i (fused pipeline, overlaps across i):
      energy: 32 matmuls (lhsT=qT[:,i,k,:], rhs=qT[:,:,k,:]) accum in PSUM
      softmax: rowmin (DVE) -> exp(mn-E) on ACT writing bf16 att +
               fused row-sum (accum_out); rg = gamma/s kept per-partition
      attT: one DMA xbar transpose of att[:,i,:] -> [128, 4(j), 128]
      out: per 512-col chunk: 4 bf16 matmuls + one DVE
           scalar_tensor_tensor: out = (psum * rg) + x  (exact fp32 x-add)
      out DMA per chunk pair.
  - att is left unnormalized; gamma/s scaling rides the final DVE op, so
    gamma=0 gives out == x exactly.
"""

import sys

import numpy as np

for _p in ("/opt/trn_rl_repo",):
    if _p not in sys.path:
        sys.path.insert(0, _p)

B, C, H, W = 8, 512, 64, 64
N = H * W  # 4096
P = 128
CT = C // P  # 4 channel tiles
KT = N // P  # 32 spatial tiles
FD = 512  # matmul free-dim / PSUM bank width (fp32)
NCH = N // FD  # 8 output column chunks
LCH = 4  # input load chunks per c-tile
LW = N // LCH  # 1024

_CACHE = {}


def _build_bass():
    import concourse.mybir as mybir
    import concourse.tile as tile
    from concourse import bacc

    fp32 = mybir.dt.float32
    bf16 = mybir.dt.bfloat16
    AX = mybir.AxisListType.X
    ALU = mybir.AluOpType
    ACT_EXP = mybir.ActivationFunctionType.Exp

    nc = bacc.Bacc(None, target_bir_lowering=False, debug=False)
    x_d = nc.dram_tensor("x", [C, N], fp32, kind="ExternalInput")
    g_d = nc.dram_tensor("gamma", [1], fp32, kind="ExternalInput")
    o_d = nc.dram_tensor("out", [C, N], fp32, kind="ExternalOutput")

    with tile.TileContext(nc) as tc:
        with (
            tc.tile_pool(name="persist", bufs=1) as persist,
            tc.tile_pool(name="stats", bufs=4) as stats,
            tc.tile_pool(name="outp", bufs=4) as outp,
            tc.tile_pool(name="epsum", bufs=2, space="PSUM") as epsum,
            tc.tile_pool(name="opsum", bufs=4, space="PSUM") as opsum,
        ):
            gam = persist.tile([P, 1], fp32)
            nc.gpsimd.dma_start(out=gam, in_=g_d[:].to_broadcast((P, 1)))

            q = persist.tile([P, CT, N], fp32)
            q_bf = persist.tile([P, CT, N], bf16)
            qT = persist.tile([P, CT, KT, P], bf16)
            att = persist.tile([P, CT, C], bf16)
            attT = persist.tile([P, CT, CT, P], bf16)

            # load fp32 in 1MB chunks, cast to bf16 on DVE, xbar-transpose.
            # Each issuing engine owns one DMA ring (FIFO), so: loads split
            # over gpsimd(SWDGE)+scalar(ACT-HWDGE), transposes split over
            # sync(SP-HWDGE)+scalar, stores on gpsimd. A transpose waiting on
            # its cast must never sit ahead of a load in the same ring.
            HW = N // 2  # 2048-wide half-tiles
            for h in range(2):
                for c in range(CT):
                    sl = slice(h * HW, (h + 1) * HW)
                    ld = nc.gpsimd if c % 2 == 0 else nc.scalar
                    ld.dma_start(out=q[:, c, sl], in_=x_d[c * P : (c + 1) * P, sl])
            for h in range(2):
                for c in range(CT):
                    sl = slice(h * HW, (h + 1) * HW)
                    nc.vector.tensor_copy(out=q_bf[:, c, sl], in_=q[:, c, sl])
                    ksl = slice(h * (HW // P), (h + 1) * (HW // P))
                    tr = nc.sync if c % 2 == 0 else nc.scalar
                    tr.dma_start_transpose(out=qT[:, c, ksl, :], in_=q_bf[:, c, sl])

            for i in range(CT):
                # ---- energy row-block i: E = q[i-block] @ q.T ----
                E = epsum.tile([P, C], fp32, name=f"E{i}", tag="E")
                for k in range(KT):
                    nc.tensor.matmul(
                        E,
                        lhsT=qT[:, i, k, :],
                        rhs=qT[:, :, k, :],
                        start=(k == 0),
                        stop=(k == KT - 1),
                    )

                # ---- softmax (unnormalized): att = exp(mn - E), s = rowsum ----
                mn = stats.tile([P, 1], fp32)
                nc.vector.tensor_reduce(out=mn, in_=E, axis=AX, op=ALU.min)
                s = stats.tile([P, 1], fp32)
                nc.scalar.activation(
                    out=att[:, i, :],
                    in_=E,
                    func=ACT_EXP,
                    bias=mn,
                    scale=-1.0,
                    accum_out=s,
                )
                rg = stats.tile([P, 1], fp32)
                nc.vector.reciprocal(out=rg, in_=s)
                nc.vector.tensor_mul(rg, rg, gam)

                # ---- attT slab i via xbar transpose ----
                tr = nc.sync if i % 2 == 0 else nc.scalar
                tr.dma_start_transpose(out=attT[:, i, :, :], in_=att[:, i, :])

                # ---- out row-block i ----
                ot = outp.tile([P, N], fp32, name="ot", tag="ot", bufs=2)
                for nh in range(NCH):
                    sl = slice(nh * FD, (nh + 1) * FD)
                    ops = opsum.tile([P, FD], fp32, name="ops", tag="ops")
                    for j in range(CT):
                        nc.tensor.matmul(
                            ops,
                            lhsT=attT[:, i, j, :],
                            rhs=q_bf[:, j, sl],
                            start=(j == 0),
                            stop=(j == CT - 1),
                        )
                    # out = (psum * gamma/s) + x, exact fp32 add of x
                    nc.vector.scalar_tensor_tensor(
                        out=ot[:, sl],
                        in0=ops,
                        scalar=rg,
                        in1=q[:, i, sl],
                        op0=ALU.mult,
                        op1=ALU.add,
                    )
                    if nh % 4 == 3:
                        osl = slice((nh - 3) * FD, (nh + 1) * FD)
                        nc.gpsimd.dma_start(
                            out=o_d[i * P : (i + 1) * P, osl], in_=ot[:, osl]
                        )

    nc.compile()
    return nc


def _get_nc():
    if "nc" not in _CACHE:
        _CACHE["nc"] = _build_bass()
    return _CACHE["nc"]


def run(x, gamma, **run_kwargs):
    """Run on 8 cores; returns (results_list, BassKernelResults)."""
    from concourse.bass_utils import run_bass_kernel_spmd

    nc = _get_nc()
    x = np.ascontiguousarray(x, dtype=np.float32)
    gamma = np.ascontiguousarray(gamma, dtype=np.float32)
    in_maps = [
        {"x": np.ascontiguousarray(x[b].reshape(C, N)), "gamma": gamma}
        for b in range(B)
    ]
    res = run_bass_kernel_spmd(nc, in_maps, core_ids=list(range(B)), **run_kwargs)
    out = np.stack([r["out"] for r in res.results]).reshape(B, C, H, W)
    return out, res


def kernel(x, gamma):
    out, _ = run(x, gamma)
    return out.astype(np.float32)


# revision 15
# speedup vs baseline: 1.3672x; 1.0743x over previous
"""CAM (channel attention) module kernel for Trainium2, 8-core data-parallel.

Computes, per batch b (one batch per NeuronCore):
    q = x[b].reshape(C, N)                  # C=512, N=4096
    E = q @ q.T                             # [C, C], symmetric
    att = softmax(rowmax(E) - E, axis=-1)   # == exp(rowmin(E)-E)/rowsum
    out = gamma * (att @ q) + x[b]

v2 design (fp32 matmul on trn2 is 2-pass LOW_HIGH emulation, ~5.5x slower
than bf16 -> do all matmuls in bf16, keep the +x and final scaling in fp32):
  - q loaded fp32 [128, 4, 4096]; cast to bf16 on DVE/ACT in 16 chunks.
  - qT built with DMA xbar transposes (bf16) into [128, 4(c), 32(k), 128],
    no PE or DVE time spent on transposition.
  - per channel-tile i (fused pipeline, overlaps across i):
      energy: 32 matmuls (lhsT=qT[:,i,k,:], rhs=qT[:,:,k,:]) accum in PSUM
      softmax: rowmin (DVE) -> exp(mn-E) on ACT writing bf16 att +
               fused row-sum (accum_out); rg = gamma/s kept per-partition
      attT: one DMA xbar transpose of att[:,i,:] -> [128, 4(j), 128]
      out: per 512-col chunk: 4 bf16 matmuls + one DVE
           scalar_tensor_tensor: out = (psum * rg) + x  (exact fp32 x-add)
      out DMA per chunk pair.
  - att is left unnormalized; gamma/s scaling rides the final DVE op, so
    gamma=0 gives out == x exactly.
"""

import sys

import numpy as np

for _p in ("/opt/trn_rl_repo",):
    if _p not in sys.path:
        sys.path.insert(0, _p)

B, C, H, W = 8, 512, 64, 64
N = H * W  # 4096
P = 128
CT = C // P  # 4 channel tiles
KT = N // P  # 32 spatial tiles
FD = 512  # matmul free-dim / PSUM bank width (fp32)
NCH = N // FD  # 8 output column chunks
LCH = 4  # input load chunks per c-tile
LW = N // LCH  # 1024

_CACHE = {}


def _build_bass():
    import concourse.mybir as mybir
    import concourse.tile as tile
    from concourse import bacc

    fp32 = mybir.dt.float32
    bf16 = mybir.dt.bfloat16
    AX = mybir.AxisListType.X
    ALU = mybir.AluOpType
    ACT_EXP = mybir.ActivationFunctionType.Exp

    nc = bacc.Bacc(None, target_bir_lowering=False, debug=False)
    x_d = nc.dram_tensor("x", [C, N], fp32, kind="ExternalInput")
    g_d = nc.dram_tensor("gamma", [1], fp32, kind="ExternalInput")
    o_d = nc.dram_tensor("out", [C, N], fp32, kind="ExternalOutput")

    with tile.TileContext(nc) as tc:
        with (
            tc.tile_pool(name="persist", bufs=1) as persist,
            tc.tile_pool(name="stats", bufs=4) as stats,
            tc.tile_pool(name="outp", bufs=4) as outp,
            tc.tile_pool(name="epsum", bufs=4, space="PSUM") as epsum,
            tc.tile_pool(name="opsum", bufs=4, space="PSUM") as opsum,
        ):
            gam = persist.tile([P, 1], fp32)
            q = persist.tile([P, CT, N], fp32)
            q_bf = persist.tile([P, CT, N], bf16)
            qT = persist.tile([P, CT, KT, P], bf16)
            att = persist.tile([P, CT, C], bf16)
            attT = persist.tile([P, CT, CT, P], bf16)

            # load fp32 in 1MB chunks, cast to bf16 on DVE, xbar-transpose.
            # Each issuing engine owns one DMA ring (FIFO), so: loads split
            # over gpsimd(SWDGE)+scalar(ACT-HWDGE), transposes split over
            # sync(SP-HWDGE)+scalar, stores on gpsimd. A transpose waiting on
            # its cast must never sit ahead of a load in the same ring.
            HW = N // 2  # 2048-wide half-tiles
            ld_rings = [nc.gpsimd, nc.scalar, nc.sync, nc.gpsimd,
                        nc.scalar, nc.sync, nc.gpsimd, nc.scalar]
            tr_rings = [nc.sync, nc.scalar] * 4
            for idx, (h, c) in enumerate([(h, c) for h in range(2) for c in range(CT)]):
                sl = slice(h * HW, (h + 1) * HW)
                ld_rings[idx].dma_start(
                    out=q[:, c, sl], in_=x_d[c * P : (c + 1) * P, sl]
                )
            nc.gpsimd.dma_start(out=gam, in_=g_d[:].to_broadcast((P, 1)))
            for idx, (h, c) in enumerate([(h, c) for h in range(2) for c in range(CT)]):
                sl = slice(h * HW, (h + 1) * HW)
                nc.vector.tensor_copy(out=q_bf[:, c, sl], in_=q[:, c, sl])
                ksl = slice(h * (HW // P), (h + 1) * (HW // P))
                tr_rings[idx].dma_start_transpose(out=qT[:, c, ksl, :], in_=q_bf[:, c, sl])

            # ---- energy, k-outer: consume each qT chunk as it lands ----
            Es = [
                epsum.tile([P, C], fp32, name=f"E{i}", tag=f"E{i}", bufs=1)
                for i in range(CT)
            ]
            for k in range(KT):
                for i in range(CT):
                    nc.tensor.matmul(
                        Es[i],
                        lhsT=qT[:, i, k, :],
                        rhs=qT[:, :, k, :],
                        start=(k == 0),
                        stop=(k == KT - 1),
                    )

            for i in range(CT):
                E = Es[i]
                # ---- softmax (unnormalized): att = exp(mn - E), s = rowsum ----
                mn = stats.tile([P, 1], fp32)
                nc.vector.tensor_reduce(out=mn, in_=E, axis=AX, op=ALU.min)
                s = stats.tile([P, 1], fp32)
                nc.scalar.activation(
                    out=att[:, i, :],
                    in_=E,
                    func=ACT_EXP,
                    bias=mn,
                    scale=-1.0,
                    accum_out=s,
                )
                rg = stats.tile([P, 1], fp32)
                nc.vector.reciprocal(out=rg, in_=s)
                nc.vector.tensor_mul(rg, rg, gam)

                # ---- attT slab i via xbar transpose ----
                tr = nc.sync if i % 2 == 0 else nc.scalar
                tr.dma_start_transpose(out=attT[:, i, :, :], in_=att[:, i, :])

                # ---- out row-block i ----
                ot = outp.tile([P, N], fp32, name="ot", tag="ot", bufs=2)
                for nh in range(NCH):
                    sl = slice(nh * FD, (nh + 1) * FD)
                    ops = opsum.tile([P, FD], fp32, name="ops", tag="ops")
                    for j in range(CT):
                        nc.tensor.matmul(
                            ops,
                            lhsT=attT[:, i, j, :],
                            rhs=q_bf[:, j, sl],
                            start=(j == 0),
                            stop=(j == CT - 1),
                        )
                    # out = (psum * gamma/s) + x, exact fp32 add of x
                    nc.vector.scalar_tensor_tensor(
                        out=ot[:, sl],
                        in0=ops,
                        scalar=rg,
                        in1=q[:, i, sl],
                        op0=ALU.mult,
                        op1=ALU.add,
                    )
                    if nh % 4 == 3:
                        osl = slice((nh - 3) * FD, (nh + 1) * FD)
                        st = nc.gpsimd if (i * 2 + nh // 4) % 2 == 0 else nc.sync
                        st.dma_start(
                            out=o_d[i * P : (i + 1) * P, osl], in_=ot[:, osl]
                        )

    nc.compile()
    return nc


def _get_nc():
    if "nc" not in _CACHE:
        _CACHE["nc"] = _build_bass()
    return _CACHE["nc"]


def run(x, gamma, **run_kwargs):
    """Run on 8 cores; returns (results_list, BassKernelResults)."""
    from concourse.bass_utils import run_bass_kernel_spmd

    nc = _get_nc()
    x = np.ascontiguousarray(x, dtype=np.float32)
    gamma = np.ascontiguousarray(gamma, dtype=np.float32)
    in_maps = [
        {"x": np.ascontiguousarray(x[b].reshape(C, N)), "gamma": gamma}
        for b in range(B)
    ]
    res = run_bass_kernel_spmd(nc, in_maps, core_ids=list(range(B)), **run_kwargs)
    out = np.stack([r["out"] for r in res.results]).reshape(B, C, H, W)
    return out, res


def kernel(x, gamma):
    out, _ = run(x, gamma)
    return out.astype(np.float32)
